# revision 19
# baseline (speedup 1.0000x reference)
"""Locally-connected layer (unshared 3x3 conv, torch-unfold semantics) on 8 trn2 cores.

out[b,o,y,x] = sum_{c,i,j} weight[o, c*9+i*3+j, y*32+x] * xpad[b, c, y+i, x+j]

Sharding: spatial over L — core r owns image rows [4r, 4r+4) (128 pixels).

v3 design (bf16, weights-stationary, N=128 moving, single slab tile):
  * Everything bf16 on the wire (tolerance 2e-2; bf16 error ~1%). PSUM fp32.
  * SBUF slab T1 [128, B*204] = [slab | slab shifted +1 col] (host-built,
    one contiguous HBM DMA).  All im2col is pure access-pattern offsets.
  * The 576-long contraction is reordered into chunks whose stationary is a
    [K, 128] host-packed weight block covering BOTH pixels of a pair
    (cols m = 64*e + o: pixel parity e, channel o).  Moving operand is
    x [K, N=128=(b, pix)] read as t13[:, :, off:off+2].  PSUM [128, (b,pix)]:
    only the e==pix halves are read out, the rest is discarded.
      q0..q2: K=128  rows [c x (i=q,j=0) | c x (i=q,j=1)]  rhs t13[0:128] @ off
      s2:     K=64   rows  c x (2,2)  (ws blob, rows 0:64) rhs t13[0:64]
      q3:     K=128  rows [c x (0,2) | c x (1,2)]          rhs t23[0:128]
    t1 = [slab | slab+1col] (both halves straight from HBM on disjoint DMA
    engines), t2 = [slab | slab+1row] (SBUF->SBUF copies; only q3 reads it,
    scheduled last in each pair for slack).  All matmuls sit at row base 0
    (mixed-base accumulation groups crash TRN2).
  * 5 matmuls / 5 ldweights per pixel pair (320 per core), all with 128-col
    stationaries and N=128 moving.  PSUM readout alternates vector/scalar by
    pair so the two engines touch different PSUM banks.  DMA instructions are
    spread across the sync/scalar/vector/gpsimd queues so descriptor
    generation and semaphore waits never serialize the weight stream.
  * Output bf16 in [psum-partition, pair, b] DRAM layout (contiguous DMA);
    host transposes to (B, O, H, W) fp32.
"""

import numpy as np
import ml_dtypes

BF16 = ml_dtypes.bfloat16

B, C, O, H, W, KS = 64, 64, 64, 32, 32, 3
L = H * W
NCORES = 8
RPC = H // NCORES            # image rows per core = 4
LC = RPC * W                 # pixels per core = 128
NP = LC // 2                 # pixel pairs per core = 64
HALO = RPC + 2               # 6 slab rows
WP = W + 2                   # padded width 34
BST = HALO * WP              # per-b free stride in the slab = 204
PG = 16                      # pairs per weight DMA group
NG = NP // PG                # weight groups = 4

_CACHE = {}


def _build_nc():
    import concourse.bass as bass
    import concourse.bacc as bacc
    import concourse.tile as tile
    from concourse import mybir

    f32 = mybir.dt.float32
    bf16 = mybir.dt.bfloat16
    nc = bacc.Bacc(
        "TRN2", target_bir_lowering=False, debug=False, num_devices=NCORES
    )
    x_d = nc.dram_tensor("x", [64, B * BST], bf16, kind="ExternalInput")
    wq_d = nc.dram_tensor("wq", [NG, 128, PG, 4, 128], bf16, kind="ExternalInput")
    ws_d = nc.dram_tensor("ws", [NG, 64, PG, 128], bf16, kind="ExternalInput")
    o_d = nc.dram_tensor("out", [128, NP, B], bf16, kind="ExternalOutput")

    with tile.TileContext(nc) as tc:
        with (
            tc.tile_pool(name="x1", bufs=1) as x1pool,
            tc.tile_pool(name="x2", bufs=1) as x2pool,
            tc.tile_pool(name="wq", bufs=2) as wpool,
            tc.tile_pool(name="ws", bufs=2) as spool,
            tc.tile_pool(name="orow", bufs=2) as opool,
            tc.tile_pool(name="ps", bufs=8, space=bass.MemorySpace.PSUM) as pspool,
        ):
            t1 = x1pool.tile([128, B * BST], bf16)
            t13 = t1[:].rearrange("p (b f) -> p b f", f=BST)
            t2 = x2pool.tile([128, B * BST], bf16)
            t23 = t2[:].rearrange("p (b f) -> p b f", f=BST)
            NF = B * BST
            # t1 = [slab | slab+1col]: lower from HBM on the scalar HWDGE
            # ring, upper straight from HBM as a flat +1-element view on the
            # vector ring (writes partitions 64:128 -> DMA engines 8-15, so
            # the two loads use disjoint engines; no inter-DMA deps).  The
            # garbage element at each b-block boundary is never read.
            nc.scalar.dma_start(t1[0:64, :], x_d[:])
            nc.scalar.dma_start(t1[64:128, 0 : NF - 1], x_d[:, 1:NF])
            # t2 = [slab | slab+1row] via SBUF->SBUF copies from t1's lower
            # half (only the q3 chunk reads t2, scheduled last in each pair).
            nc.gpsimd.dma_start(t2[0:64, :], t1[0:64, :])
            nc.gpsimd.dma_start(
                t2[64:128, 0 : NF - WP], t1[0:64, WP:NF]
            )

            for g in range(NG):
                wt = wpool.tile([128, PG, 4, 128], bf16)
                st = spool.tile([64, PG, 128], bf16)
                nc.sync.dma_start(wt[:], wq_d[g])
                nc.scalar.dma_start(st[:], ws_d[g])
                orow = opool.tile([128, PG, B], bf16)
                for tt in range(PG):
                    t = g * PG + tt
                    y, x0 = (2 * t) // W, (2 * t) % W
                    ps = pspool.tile([128, B, 2], f32)
                    for q in range(3):
                        off = (y + q) * WP + x0
                        nc.tensor.matmul(
                            ps[:], wt[:, tt, q, :], t13[:, :, off : off + 2],
                            start=(q == 0), stop=False,
                        )
                    offs = (y + 2) * WP + x0 + 2
                    nc.tensor.matmul(
                        ps[:], st[:, tt, :], t13[0:64, :, offs : offs + 2],
                        start=False, stop=False,
                    )
                    offq3 = y * WP + x0 + 2
                    nc.tensor.matmul(
                        ps[:], wt[:, tt, 3, :], t23[:, :, offq3 : offq3 + 2],
                        start=False, stop=True,
                    )
                    if tt % 2 == 0:
                        nc.vector.tensor_copy(orow[0:64, tt, :], ps[0:64, :, 0])
                        nc.vector.tensor_copy(orow[64:128, tt, :], ps[64:128, :, 1])
                    else:
                        nc.scalar.copy(orow[0:64, tt, :], ps[0:64, :, 0])
                        nc.scalar.copy(orow[64:128, tt, :], ps[64:128, :, 1])
                nc.gpsimd.dma_start(o_d[:, g * PG : (g + 1) * PG, :], orow[:])
    nc.compile()
    return nc


def _get_nc():
    if "nc" not in _CACHE:
        _CACHE["nc"] = _build_nc()
    return _CACHE["nc"]


def _pack_x(x):
    """Per core: [64, B*BST] bf16 lower slab (the +1-col shifted upper half
    of the on-chip tile is built by an SBUF->SBUF DMA inside the kernel)."""
    xpad = np.pad(x, ((0, 0), (0, 0), (1, 1), (1, 1)))
    xpad = np.ascontiguousarray(xpad.transpose(1, 0, 2, 3))  # [C, B, 34, 34]
    outs = []
    for r in range(NCORES):
        slab = xpad[:, :, RPC * r : RPC * r + HALO, :].reshape(C, B, BST)
        outs.append(np.ascontiguousarray(slab.astype(BF16).reshape(64, B * BST)))
    return outs


def _pack_w(weight):
    """Chunked-contraction weight blobs, already in SBUF layout.

    wq: [core, NG, p=(half, c), tt, q, m=(e, o)]
        q0..q2: half = j (shift (q, j));  q3: half 0 = (0,2), half 1 = (1,2)
    ws: [core, NG, c, tt, m=(e, o)]   (the (2,2) single)
    """
    w5 = weight.reshape(O, C, KS, KS, L)
    low = np.stack(
        [w5[:, :, 0, 0], w5[:, :, 1, 0], w5[:, :, 2, 0], w5[:, :, 0, 2]], axis=0
    )
    up = np.stack(
        [w5[:, :, 0, 1], w5[:, :, 1, 1], w5[:, :, 2, 1], w5[:, :, 1, 2]], axis=0
    )
    wq = np.stack([low, up], axis=1)          # [q, half, O, C, L]
    wq = wq.reshape(4, 2, O, C, NCORES, NG, PG, 2)
    # -> [core, g, half, c, tt, q, e, o]
    wq = wq.transpose(4, 5, 1, 3, 6, 0, 7, 2)
    wq = np.ascontiguousarray(wq, dtype=BF16).reshape(NCORES, NG, 128, PG, 4, 128)

    ws = w5[:, :, 2, 2].reshape(O, C, NCORES, NG, PG, 2)
    ws = ws.transpose(2, 3, 1, 4, 5, 0)       # [core, g, c, tt, e, o]
    ws = np.ascontiguousarray(ws, dtype=BF16).reshape(NCORES, NG, 64, PG, 128)
    return wq, ws


def kernel(x, weight, bias, _trace=False, _trace_kwargs=None):
    from concourse.bass_utils import run_bass_kernel_spmd

    x = np.asarray(x, dtype=np.float32)
    weight = np.asarray(weight, dtype=np.float32)
    bias = np.asarray(bias, dtype=np.float32)

    nc = _get_nc()
    xs = _pack_x(x)
    wq, ws = _pack_w(weight)
    in_maps = [
        {"x": xs[r], "wq": wq[r], "ws": ws[r]} for r in range(NCORES)
    ]
    res = run_bass_kernel_spmd(
        nc, in_maps, list(range(NCORES)),
        trace=_trace, **(_trace_kwargs or {}),
    )
    # out[r]: [p=(e,o), t, b] bf16 -> [b, o, l=128r+2t+e]
    parts = []
    for r in range(NCORES):
        arr = res.results[r]["out"].astype(np.float32)
        arr = arr.reshape(2, O, NP, B).transpose(3, 1, 2, 0)  # [b, o, t, e]
        parts.append(arr.reshape(B, O, LC))
    out = np.concatenate(parts, axis=2).reshape(B, O, H, W)
    if np.any(bias):
        out = out + bias.reshape(1, O, H, W)
    if _trace:
        _CACHE["last_result"] = res
    return np.ascontiguousarray(out.astype(np.float32))


# revision 25
# speedup vs baseline: 1.3858x; 1.3858x over previous
"""Locally-connected layer (unshared 3x3 conv, torch-unfold semantics) on 8 trn2 cores.

out[b,o,y,x] = sum_{c,i,j} weight[o, c*9+i*3+j, y*32+x] * xpad[b, c, y+i, x+j]

Sharding: spatial over L — core r owns image rows [4r, 4r+4) (128 pixels).

v3 design (bf16, weights-stationary, N=128 moving, single slab tile):
  * Everything bf16 on the wire (tolerance 2e-2; bf16 error ~1%). PSUM fp32.
  * SBUF slab T1 [128, B*204] = [slab | slab shifted +1 col] (host-built,
    one contiguous HBM DMA).  All im2col is pure access-pattern offsets.
  * The 576-long contraction is reordered into chunks whose stationary is a
    [K, 128] host-packed weight block covering BOTH pixels of a pair
    (cols m = 64*e + o: pixel parity e, channel o).  Moving operand is
    x [K, N=128=(b, pix)] read as t13[:, :, off:off+2].  PSUM [128, (b,pix)]:
    only the e==pix halves are read out, the rest is discarded.
      q0..q2: K=128  rows [c x (i=q,j=0) | c x (i=q,j=1)]  rhs t13[0:128] @ off
      s2:     K=64   rows  c x (2,2)  (ws blob, rows 0:64) rhs t13[0:64]
      q3:     K=128  rows [c x (0,2) | c x (1,2)]          rhs t23[0:128]
    t1 = [slab | slab+1col] (both halves straight from HBM on disjoint DMA
    engines), t2 = [slab | slab+1row] (SBUF->SBUF copies; only q3 reads it,
    scheduled last in each pair for slack).  All matmuls sit at row base 0
    (mixed-base accumulation groups crash TRN2).
  * 5 matmuls / 5 ldweights per pixel pair (320 per core), all with 128-col
    stationaries and N=128 moving.  PSUM readout alternates vector/scalar by
    pair so the two engines touch different PSUM banks.  DMA instructions are
    spread across the sync/scalar/vector/gpsimd queues so descriptor
    generation and semaphore waits never serialize the weight stream.
  * Output bf16 in [psum-partition, pair, b] DRAM layout (contiguous DMA);
    host transposes to (B, O, H, W) fp32.
"""

import numpy as np
import ml_dtypes

BF16 = ml_dtypes.bfloat16

B, C, O, H, W, KS = 64, 64, 64, 32, 32, 3
L = H * W
NCORES = 8
RPC = H // NCORES            # image rows per core = 4
LC = RPC * W                 # pixels per core = 128
NP = LC // 2                 # pixel pairs per core = 64
HALO = RPC + 2               # 6 slab rows
WP = W + 2                   # padded width 34
BST = HALO * WP              # per-b free stride in the slab = 204
PG = 8                       # pairs per weight DMA group
NG = NP // PG                # weight groups = 8

_CACHE = {}


def _build_nc():
    import concourse.bass as bass
    import concourse.bacc as bacc
    import concourse.tile as tile
    from concourse import mybir

    f32 = mybir.dt.float32
    bf16 = mybir.dt.bfloat16
    nc = bacc.Bacc(
        "TRN2", target_bir_lowering=False, debug=False, num_devices=NCORES
    )
    x_d = nc.dram_tensor("x", [128, B * BST], bf16, kind="ExternalInput")
    wq_d = nc.dram_tensor("wq", [NG, 128, PG, 4, 128], bf16, kind="ExternalInput")
    ws_d = nc.dram_tensor("ws", [64, NP, 128], bf16, kind="ExternalInput")
    o_d = nc.dram_tensor("out", [128, NP, B], bf16, kind="ExternalOutput")

    with tile.TileContext(nc) as tc:
        with (
            tc.tile_pool(name="x1", bufs=1) as x1pool,
            tc.tile_pool(name="x2", bufs=1) as x2pool,
            tc.tile_pool(name="wq", bufs=4) as wpool,
            tc.tile_pool(name="ws", bufs=1) as spool,
            tc.tile_pool(name="orow", bufs=2) as opool,
            tc.tile_pool(name="ps", bufs=8, space=bass.MemorySpace.PSUM) as pspool,
        ):
            t1 = x1pool.tile([128, B * BST], bf16)
            t13 = t1[:].rearrange("p (b f) -> p b f", f=BST)
            t2 = x2pool.tile([128, B * BST], bf16)
            t23 = t2[:].rearrange("p (b f) -> p b f", f=BST)
            NF = B * BST
            # t1 = [slab | slab+1col] host-built: two free-dim-split DMAs on
            # the scalar HWDGE ring (each spans all 16 DMA engines, no deps).
            nc.scalar.dma_start(t1[:, 0 : NF // 2], x_d[:, 0 : NF // 2])
            nc.scalar.dma_start(t1[:, NF // 2 : NF], x_d[:, NF // 2 : NF])
            # all singles' weights in one upfront DMA
            st = spool.tile([64, NP, 128], bf16)
            nc.scalar.dma_start(st[:], ws_d[:])
            # t2 = [slab | slab+1row] via SBUF->SBUF copies from t1's lower
            # half (only the q3 chunk reads t2, scheduled last in each pair).
            nc.gpsimd.dma_start(t2[0:64, :], t1[0:64, :])
            nc.gpsimd.dma_start(
                t2[64:128, 0 : NF - WP], t1[0:64, WP:NF]
            )

            for g in range(NG):
                wt = wpool.tile([128, PG, 4, 128], bf16)
                nc.sync.dma_start(wt[:], wq_d[g])
                orow = opool.tile([128, PG, B], bf16)
                for tt in range(PG):
                    t = g * PG + tt
                    y, x0 = (2 * t) // W, (2 * t) % W
                    ps = pspool.tile([128, B, 2], f32)
                    for q in range(3):
                        off = (y + q) * WP + x0
                        nc.tensor.matmul(
                            ps[:], wt[:, tt, q, :], t13[:, :, off : off + 2],
                            start=(q == 0), stop=False,
                        )
                    offs = (y + 2) * WP + x0 + 2
                    nc.tensor.matmul(
                        ps[:], st[:, t, :], t13[0:64, :, offs : offs + 2],
                        start=False, stop=False,
                    )
                    offq3 = y * WP + x0 + 2
                    nc.tensor.matmul(
                        ps[:], wt[:, tt, 3, :], t23[:, :, offq3 : offq3 + 2],
                        start=False, stop=True,
                    )
                    if tt % 2 == 0:
                        nc.vector.tensor_copy(orow[0:64, tt, :], ps[0:64, :, 0])
                        nc.vector.tensor_copy(orow[64:128, tt, :], ps[64:128, :, 1])
                    else:
                        nc.scalar.copy(orow[0:64, tt, :], ps[0:64, :, 0])
                        nc.scalar.copy(orow[64:128, tt, :], ps[64:128, :, 1])
                nc.gpsimd.dma_start(o_d[:, g * PG : (g + 1) * PG, :], orow[:])
    nc.compile()
    return nc


def _get_nc():
    if "nc" not in _CACHE:
        _CACHE["nc"] = _build_nc()
    return _CACHE["nc"]


def _pack_x(x):
    """Per core: [128, B*BST] bf16 = [slab | slab shifted +1 col]."""
    xpad = np.pad(x, ((0, 0), (0, 0), (1, 1), (1, 1)))
    xpad = np.ascontiguousarray(xpad.transpose(1, 0, 2, 3))  # [C, B, 34, 34]
    outs = []
    for r in range(NCORES):
        slab = xpad[:, :, RPC * r : RPC * r + HALO, :].reshape(C, B * BST)
        up = np.zeros_like(slab)
        up[:, : B * BST - 1] = slab[:, 1:]
        t1 = np.concatenate([slab, up], axis=0).astype(BF16)
        outs.append(np.ascontiguousarray(t1))
    return outs


def _pack_w(weight):
    """Chunked-contraction weight blobs, already in SBUF layout.

    wq: [core, NG, p=(half, c), tt, q, m=(e, o)]
        q0..q2: half = j (shift (q, j));  q3: half 0 = (0,2), half 1 = (1,2)
    ws: [core, c, t, m=(e, o)]   (the (2,2) single, all pairs in one blob)
    """
    w5 = weight.reshape(O, C, KS, KS, L)
    low = np.stack(
        [w5[:, :, 0, 0], w5[:, :, 1, 0], w5[:, :, 2, 0], w5[:, :, 0, 2]], axis=0
    )
    up = np.stack(
        [w5[:, :, 0, 1], w5[:, :, 1, 1], w5[:, :, 2, 1], w5[:, :, 1, 2]], axis=0
    )
    wq = np.stack([low, up], axis=1)          # [q, half, O, C, L]
    wq = wq.reshape(4, 2, O, C, NCORES, NG, PG, 2)
    # -> [core, g, half, c, tt, q, e, o]
    wq = wq.transpose(4, 5, 1, 3, 6, 0, 7, 2)
    wq = np.ascontiguousarray(wq, dtype=BF16).reshape(NCORES, NG, 128, PG, 4, 128)

    ws = w5[:, :, 2, 2].reshape(O, C, NCORES, NP, 2)
    ws = ws.transpose(2, 1, 3, 4, 0)          # [core, c, t, e, o]
    ws = np.ascontiguousarray(ws, dtype=BF16).reshape(NCORES, 64, NP, 128)
    return wq, ws


def kernel(x, weight, bias, _trace=False, _trace_kwargs=None):
    from concourse.bass_utils import run_bass_kernel_spmd

    x = np.asarray(x, dtype=np.float32)
    weight = np.asarray(weight, dtype=np.float32)
    bias = np.asarray(bias, dtype=np.float32)

    nc = _get_nc()
    xs = _pack_x(x)
    wq, ws = _pack_w(weight)
    in_maps = [
        {"x": xs[r], "wq": wq[r], "ws": ws[r]} for r in range(NCORES)
    ]
    res = run_bass_kernel_spmd(
        nc, in_maps, list(range(NCORES)),
        trace=_trace, **(_trace_kwargs or {}),
    )
    # out[r]: [p=(e,o), t, b] bf16 -> [b, o, l=128r+2t+e]
    parts = []
    for r in range(NCORES):
        arr = res.results[r]["out"].astype(np.float32)
        arr = arr.reshape(2, O, NP, B).transpose(3, 1, 2, 0)  # [b, o, t, e]
        parts.append(arr.reshape(B, O, LC))
    out = np.concatenate(parts, axis=2).reshape(B, O, H, W)
    if np.any(bias):
        out = out + bias.reshape(1, O, H, W)
    if _trace:
        _CACHE["last_result"] = res
    return np.ascontiguousarray(out.astype(np.float32))


# revision 26
# speedup vs baseline: 1.8413x; 1.3287x over previous
"""Locally-connected layer (unshared 3x3 conv, torch-unfold semantics) on 8 trn2 cores.

out[b,o,y,x] = sum_{c,i,j} weight[o, c*9+i*3+j, y*32+x] * xpad[b, c, y+i, x+j]

Sharding: spatial over L — core r owns image rows [4r, 4r+4) (128 pixels).

v8 design (bf16, weights-stationary, N=128 moving, row-major slab):
  * Everything bf16 on the wire (tolerance 2e-2; measured error ~0.29%).
    PSUM accumulates fp32.
  * SBUF slab T1 [128, (row, b, w)] = [slab | slab shifted +1 col], host-built
    row-major so the first weight groups only need slab rows 0..3: the x load
    is split [rows 0:4 | rows 4:6] and compute starts after the first part.
  * The 576-long contraction is reordered into 6 chunks; the stationary of
    each is a [K, 128] host-packed weight block covering BOTH pixels of a
    pair (cols m = 64*e + o).  Moving operand is x [K, N=128=(b, pix)] read
    as t1r[:, row, :, x0:x0+2].  PSUM [128, (b,pix)]: only the e==pix halves
    are read out.
      q0..q2: K=128  rows [c x (i=q,j=0) | c x (i=q,j=1)]
      s0..s2: K=64   rows  c x (s,2)  (ws blob, rows 0:64)
    All matmuls sit at row base 0 (mixed-base accumulation groups crash TRN2).
  * 6 matmuls / 6 ldweights per pixel pair (384 per core), 128-col
    stationaries, N=128 moving.  PSUM readout alternates vector/scalar by
    pair so the two engines touch different PSUM banks.
  * DMA budget is the binding resource (~340 GB/s/core aggregate under
    8-core load): total moved = 3.34 (x) + 9.44 (w) + 1.05 (out bf16) MB.
    Weight stream owns the sync HWDGE queue (nothing ever blocks it), x is
    on scalar, output DMAs on gpsimd.
  * Output bf16 in [psum-partition, pair, b] DRAM layout (contiguous DMA);
    host transposes to (B, O, H, W) fp32.
"""

import numpy as np
import ml_dtypes

BF16 = ml_dtypes.bfloat16

B, C, O, H, W, KS = 64, 64, 64, 32, 32, 3
L = H * W
NCORES = 8
RPC = H // NCORES            # image rows per core = 4
LC = RPC * W                 # pixels per core = 128
NP = LC // 2                 # pixel pairs per core = 64
HALO = RPC + 2               # 6 slab rows
WP = W + 2                   # padded width 34
PG = 8                       # pairs per weight DMA group
NG = NP // PG                # weight groups = 8

_CACHE = {}


def _build_nc():
    import concourse.bass as bass
    import concourse.bacc as bacc
    import concourse.tile as tile
    from concourse import mybir

    f32 = mybir.dt.float32
    bf16 = mybir.dt.bfloat16
    nc = bacc.Bacc(
        "TRN2", target_bir_lowering=False, debug=False, num_devices=NCORES
    )
    x_d = nc.dram_tensor("x", [128, HALO, B, WP], bf16, kind="ExternalInput")
    wq_d = nc.dram_tensor("wq", [NG, 128, PG, 3, 128], bf16, kind="ExternalInput")
    ws_d = nc.dram_tensor("ws", [NG, 64, PG, 3, 128], bf16, kind="ExternalInput")
    o_d = nc.dram_tensor("out", [128, NP, B], bf16, kind="ExternalOutput")

    with tile.TileContext(nc) as tc:
        with (
            tc.tile_pool(name="x1", bufs=1) as x1pool,
            tc.tile_pool(name="wq", bufs=4) as wpool,
            tc.tile_pool(name="ws", bufs=4) as spool,
            tc.tile_pool(name="orow", bufs=2) as opool,
            tc.tile_pool(name="ps", bufs=8, space=bass.MemorySpace.PSUM) as pspool,
        ):
            t1 = x1pool.tile([128, HALO * B * WP], bf16)
            t1r = t1[:].rearrange("p (r b w) -> p r b w", r=HALO, b=B)
            # row-major slab: rows 0..3 land first, compute starts under the
            # tail of the x transfer.
            nc.scalar.dma_start(t1r[:, 0:4], x_d[:, 0:4])
            nc.scalar.dma_start(t1r[:, 4:6], x_d[:, 4:6])

            for g in range(NG):
                wt = wpool.tile([128, PG, 3, 128], bf16)
                st = spool.tile([64, PG, 3, 128], bf16)
                nc.sync.dma_start(wt[:], wq_d[g])
                nc.sync.dma_start(st[:], ws_d[g])
                orow = opool.tile([128, PG, B], bf16)
                for tt in range(PG):
                    t = g * PG + tt
                    y, x0 = (2 * t) // W, (2 * t) % W
                    ps = pspool.tile([128, B, 2], f32)
                    for q in range(3):
                        nc.tensor.matmul(
                            ps[:], wt[:, tt, q, :],
                            t1r[:, y + q, :, x0 : x0 + 2],
                            start=(q == 0), stop=False,
                        )
                    for s in range(3):
                        nc.tensor.matmul(
                            ps[:], st[:, tt, s, :],
                            t1r[0:64, y + s, :, x0 + 2 : x0 + 4],
                            start=False, stop=(s == 2),
                        )
                    if tt % 2 == 0:
                        nc.vector.tensor_copy(orow[0:64, tt, :], ps[0:64, :, 0])
                        nc.vector.tensor_copy(orow[64:128, tt, :], ps[64:128, :, 1])
                    else:
                        nc.scalar.copy(orow[0:64, tt, :], ps[0:64, :, 0])
                        nc.scalar.copy(orow[64:128, tt, :], ps[64:128, :, 1])
                nc.gpsimd.dma_start(o_d[:, g * PG : (g + 1) * PG, :], orow[:])
    nc.compile()
    return nc


def _get_nc():
    if "nc" not in _CACHE:
        _CACHE["nc"] = _build_nc()
    return _CACHE["nc"]


def _pack_x(x):
    """Per core: [128, HALO, B, WP] bf16 = [slab | slab shifted +1 col],
    row-major so row blocks stream independently."""
    xpad = np.pad(x, ((0, 0), (0, 0), (1, 1), (1, 1)))
    xpad = np.ascontiguousarray(xpad.transpose(1, 0, 2, 3))  # [C, B, 34, 34]
    outs = []
    for r in range(NCORES):
        slab = xpad[:, :, RPC * r : RPC * r + HALO, :]       # [C, B, 6, 34]
        slab = slab.transpose(0, 2, 1, 3)                    # [C, 6, B, 34]
        up = np.zeros_like(slab)
        up[..., : WP - 1] = slab[..., 1:]
        t1 = np.concatenate([slab, up], axis=0).astype(BF16)
        outs.append(np.ascontiguousarray(t1))
    return outs


def _pack_w(weight):
    """Chunked-contraction weight blobs, already in SBUF layout.

    wq: [core, NG, p=(j, c), tt, q, m=(e, o)]   (pair chunks, shifts (q, j))
    ws: [core, NG, c, tt, s, m=(e, o)]          (singles, shifts (s, 2))
    """
    w5 = weight.reshape(O, C, KS, KS, L)
    low = np.stack([w5[:, :, 0, 0], w5[:, :, 1, 0], w5[:, :, 2, 0]], axis=0)
    up = np.stack([w5[:, :, 0, 1], w5[:, :, 1, 1], w5[:, :, 2, 1]], axis=0)
    wq = np.stack([low, up], axis=1)          # [q, j, O, C, L]
    wq = wq.reshape(3, 2, O, C, NCORES, NG, PG, 2)
    # -> [core, g, j, c, tt, q, e, o]
    wq = wq.transpose(4, 5, 1, 3, 6, 0, 7, 2)
    wq = np.ascontiguousarray(wq, dtype=BF16).reshape(NCORES, NG, 128, PG, 3, 128)

    ws = np.stack([w5[:, :, 0, 2], w5[:, :, 1, 2], w5[:, :, 2, 2]], axis=0)
    ws = ws.reshape(3, O, C, NCORES, NG, PG, 2)
    ws = ws.transpose(3, 4, 2, 5, 0, 6, 1)    # [core, g, c, tt, s, e, o]
    ws = np.ascontiguousarray(ws, dtype=BF16).reshape(NCORES, NG, 64, PG, 3, 128)
    return wq, ws


def kernel(x, weight, bias, _trace=False, _trace_kwargs=None):
    from concourse.bass_utils import run_bass_kernel_spmd

    x = np.asarray(x, dtype=np.float32)
    weight = np.asarray(weight, dtype=np.float32)
    bias = np.asarray(bias, dtype=np.float32)

    nc = _get_nc()
    xs = _pack_x(x)
    wq, ws = _pack_w(weight)
    in_maps = [
        {"x": xs[r], "wq": wq[r], "ws": ws[r]} for r in range(NCORES)
    ]
    res = run_bass_kernel_spmd(
        nc, in_maps, list(range(NCORES)),
        trace=_trace, **(_trace_kwargs or {}),
    )
    # out[r]: [p=(e,o), t, b] bf16 -> [b, o, l=128r+2t+e]
    parts = []
    for r in range(NCORES):
        arr = res.results[r]["out"].astype(np.float32)
        arr = arr.reshape(2, O, NP, B).transpose(3, 1, 2, 0)  # [b, o, t, e]
        parts.append(arr.reshape(B, O, LC))
    out = np.concatenate(parts, axis=2).reshape(B, O, H, W)
    if np.any(bias):
        out = out + bias.reshape(1, O, H, W)
    if _trace:
        _CACHE["last_result"] = res
    return np.ascontiguousarray(out.astype(np.float32))


# revision 30
# speedup vs baseline: 1.8657x; 1.0133x over previous
"""Locally-connected layer (unshared 3x3 conv, torch-unfold semantics) on 8 trn2 cores.

out[b,o,y,x] = sum_{c,i,j} weight[o, c*9+i*3+j, y*32+x] * xpad[b, c, y+i, x+j]

Sharding: spatial over L — core r owns image rows [4r, 4r+4) (128 pixels).

v8 design (bf16, weights-stationary, N=128 moving, row-major slab):
  * Everything bf16 on the wire (tolerance 2e-2; measured error ~0.29%).
    PSUM accumulates fp32.
  * SBUF slab T1 [128, (row, b, w)] = [slab | slab shifted +1 col], host-built
    row-major so the first weight groups only need slab rows 0..3: the x load
    is split [rows 0:4 | rows 4:6] and compute starts after the first part.
  * The 576-long contraction is reordered into 6 chunks; the stationary of
    each is a [K, 128] host-packed weight block covering BOTH pixels of a
    pair (cols m = 64*e + o).  Moving operand is x [K, N=128=(b, pix)] read
    as t1r[:, row, :, x0:x0+2].  PSUM [128, (b,pix)]: only the e==pix halves
    are read out.
      q0..q2: K=128  rows [c x (i=q,j=0) | c x (i=q,j=1)]
      s0..s2: K=64   rows  c x (s,2)  (ws blob, rows 0:64)
    All matmuls sit at row base 0 (mixed-base accumulation groups crash TRN2).
  * 6 matmuls / 6 ldweights per pixel pair (384 per core), 128-col
    stationaries, N=128 moving.  PSUM readout alternates vector/scalar by
    pair so the two engines touch different PSUM banks.
  * DMA budget is the binding resource (~340 GB/s/core aggregate under
    8-core load): total moved = 3.34 (x) + 9.44 (w) + 1.05 (out bf16) MB.
    Weight stream owns the sync HWDGE queue (nothing ever blocks it), x is
    on scalar, output DMAs on gpsimd.
  * Output bf16 in [psum-partition, pair, b] DRAM layout (contiguous DMA);
    host transposes to (B, O, H, W) fp32.
"""

import numpy as np
import ml_dtypes

BF16 = ml_dtypes.bfloat16

B, C, O, H, W, KS = 64, 64, 64, 32, 32, 3
L = H * W
NCORES = 8
RPC = H // NCORES            # image rows per core = 4
LC = RPC * W                 # pixels per core = 128
NP = LC // 2                 # pixel pairs per core = 64
HALO = RPC + 2               # 6 slab rows
WP = W + 2                   # padded width 34
PG = 8                       # pairs per weight DMA group
NG = NP // PG                # weight groups = 8

_CACHE = {}


def _build_nc():
    import concourse.bass as bass
    import concourse.bacc as bacc
    import concourse.tile as tile
    from concourse import mybir

    f32 = mybir.dt.float32
    bf16 = mybir.dt.bfloat16
    nc = bacc.Bacc(
        "TRN2", target_bir_lowering=False, debug=False, num_devices=NCORES
    )
    x_d = nc.dram_tensor("x", [128, HALO, B, WP], bf16, kind="ExternalInput")
    wq_d = nc.dram_tensor("wq", [NG, 128, PG, 3, 128], bf16, kind="ExternalInput")
    ws_d = nc.dram_tensor("ws", [NG, 64, PG, 3, 128], bf16, kind="ExternalInput")
    o_d = nc.dram_tensor("out", [128, NP, B], bf16, kind="ExternalOutput")

    with tile.TileContext(nc) as tc:
        with (
            tc.tile_pool(name="x1", bufs=1) as x1pool,
            tc.tile_pool(name="wq", bufs=4) as wpool,
            tc.tile_pool(name="ws", bufs=4) as spool,
            tc.tile_pool(name="orow", bufs=2) as opool,
            tc.tile_pool(name="ps", bufs=7, space=bass.MemorySpace.PSUM) as pspool,
            tc.tile_pool(name="psw", bufs=1, space=bass.MemorySpace.PSUM) as pswpool,
        ):
            t1 = x1pool.tile([128, HALO * B * WP], bf16)
            t1r = t1[:].rearrange("p (r b w) -> p r b w", r=HALO, b=B)
            # row-major slab: rows 0..2 (all of y=0 needs) land first, compute
            # starts under the tail of the x transfer.
            nc.scalar.dma_start(t1r[:, 0:3], x_d[:, 0:3])
            nc.scalar.dma_start(t1r[:, 3:6], x_d[:, 3:6])

            # PE warmup: dummy matmuls while the first DMAs stream, so the
            # HAM clock gate is at 8/8 when the real matmul stream begins.
            scr = x1pool.tile([128, 256], bf16)
            nc.vector.memzero(scr[:])
            psw = pswpool.tile([64, 256], f32)
            for _ in range(24):
                nc.tensor.matmul(psw[:], scr[:, 0:64], scr[:], start=True, stop=True)

            for g in range(NG):
                wt = wpool.tile([128, PG, 3, 128], bf16)
                st = spool.tile([64, PG, 3, 128], bf16)
                nc.sync.dma_start(wt[:], wq_d[g])
                nc.sync.dma_start(st[:], ws_d[g])
                orow = opool.tile([128, PG, B], bf16)
                for tt in range(PG):
                    t = g * PG + tt
                    y, x0 = (2 * t) // W, (2 * t) % W
                    ps = pspool.tile([128, B, 2], f32)
                    for q in range(3):
                        nc.tensor.matmul(
                            ps[:], wt[:, tt, q, :],
                            t1r[:, y + q, :, x0 : x0 + 2],
                            start=(q == 0), stop=False,
                        )
                    for s in range(3):
                        nc.tensor.matmul(
                            ps[:], st[:, tt, s, :],
                            t1r[0:64, y + s, :, x0 + 2 : x0 + 4],
                            start=False, stop=(s == 2),
                        )
                    if tt % 2 == 0:
                        nc.vector.tensor_copy(orow[0:64, tt, :], ps[0:64, :, 0])
                        nc.vector.tensor_copy(orow[64:128, tt, :], ps[64:128, :, 1])
                    else:
                        nc.scalar.copy(orow[0:64, tt, :], ps[0:64, :, 0])
                        nc.scalar.copy(orow[64:128, tt, :], ps[64:128, :, 1])
                nc.scalar.dma_start(o_d[:, g * PG : (g + 1) * PG, :], orow[:])
    nc.compile()
    return nc


def _get_nc():
    if "nc" not in _CACHE:
        _CACHE["nc"] = _build_nc()
    return _CACHE["nc"]


def _pack_x(x):
    """Per core: [128, HALO, B, WP] bf16 = [slab | slab shifted +1 col],
    row-major so row blocks stream independently."""
    xpad = np.pad(x, ((0, 0), (0, 0), (1, 1), (1, 1)))
    xpad = np.ascontiguousarray(xpad.transpose(1, 0, 2, 3))  # [C, B, 34, 34]
    outs = []
    for r in range(NCORES):
        slab = xpad[:, :, RPC * r : RPC * r + HALO, :]       # [C, B, 6, 34]
        slab = slab.transpose(0, 2, 1, 3)                    # [C, 6, B, 34]
        up = np.zeros_like(slab)
        up[..., : WP - 1] = slab[..., 1:]
        t1 = np.concatenate([slab, up], axis=0).astype(BF16)
        outs.append(np.ascontiguousarray(t1))
    return outs


def _pack_w(weight):
    """Chunked-contraction weight blobs, already in SBUF layout.

    wq: [core, NG, p=(j, c), tt, q, m=(e, o)]   (pair chunks, shifts (q, j))
    ws: [core, NG, c, tt, s, m=(e, o)]          (singles, shifts (s, 2))
    """
    w5 = weight.reshape(O, C, KS, KS, L)
    low = np.stack([w5[:, :, 0, 0], w5[:, :, 1, 0], w5[:, :, 2, 0]], axis=0)
    up = np.stack([w5[:, :, 0, 1], w5[:, :, 1, 1], w5[:, :, 2, 1]], axis=0)
    wq = np.stack([low, up], axis=1)          # [q, j, O, C, L]
    wq = wq.reshape(3, 2, O, C, NCORES, NG, PG, 2)
    # -> [core, g, j, c, tt, q, e, o]
    wq = wq.transpose(4, 5, 1, 3, 6, 0, 7, 2)
    wq = np.ascontiguousarray(wq, dtype=BF16).reshape(NCORES, NG, 128, PG, 3, 128)

    ws = np.stack([w5[:, :, 0, 2], w5[:, :, 1, 2], w5[:, :, 2, 2]], axis=0)
    ws = ws.reshape(3, O, C, NCORES, NG, PG, 2)
    ws = ws.transpose(3, 4, 2, 5, 0, 6, 1)    # [core, g, c, tt, s, e, o]
    ws = np.ascontiguousarray(ws, dtype=BF16).reshape(NCORES, NG, 64, PG, 3, 128)
    return wq, ws


def kernel(x, weight, bias, _trace=False, _trace_kwargs=None):
    from concourse.bass_utils import run_bass_kernel_spmd

    x = np.asarray(x, dtype=np.float32)
    weight = np.asarray(weight, dtype=np.float32)
    bias = np.asarray(bias, dtype=np.float32)

    nc = _get_nc()
    xs = _pack_x(x)
    wq, ws = _pack_w(weight)
    in_maps = [
        {"x": xs[r], "wq": wq[r], "ws": ws[r]} for r in range(NCORES)
    ]
    res = run_bass_kernel_spmd(
        nc, in_maps, list(range(NCORES)),
        trace=_trace, **(_trace_kwargs or {}),
    )
    # out[r]: [p=(e,o), t, b] bf16 -> [b, o, l=128r+2t+e]
    parts = []
    for r in range(NCORES):
        arr = res.results[r]["out"].astype(np.float32)
        arr = arr.reshape(2, O, NP, B).transpose(3, 1, 2, 0)  # [b, o, t, e]
        parts.append(arr.reshape(B, O, LC))
    out = np.concatenate(parts, axis=2).reshape(B, O, H, W)
    if np.any(bias):
        out = out + bias.reshape(1, O, H, W)
    if _trace:
        _CACHE["last_result"] = res
    return np.ascontiguousarray(out.astype(np.float32))


# revision 33
# speedup vs baseline: 1.9157x; 1.0268x over previous
"""Locally-connected layer (unshared 3x3 conv, torch-unfold semantics) on 8 trn2 cores.

out[b,o,y,x] = sum_{c,i,j} weight[o, c*9+i*3+j, y*32+x] * xpad[b, c, y+i, x+j]

Sharding: spatial over L — core r owns image rows [4r, 4r+4) (128 pixels).

v8 design (bf16, weights-stationary, N=128 moving, row-major slab):
  * Everything bf16 on the wire (tolerance 2e-2; measured error ~0.29%).
    PSUM accumulates fp32.
  * SBUF slab T1 [128, (row, b, w)] = [slab | slab shifted +1 col], host-built
    row-major so the first weight groups only need slab rows 0..3: the x load
    is split [rows 0:4 | rows 4:6] and compute starts after the first part.
  * The 576-long contraction is reordered into 6 chunks; the stationary of
    each is a [K, 128] host-packed weight block covering BOTH pixels of a
    pair (cols m = 64*e + o).  Moving operand is x [K, N=128=(b, pix)] read
    as t1r[:, row, :, x0:x0+2].  PSUM [128, (b,pix)]: only the e==pix halves
    are read out.
      q0..q2: K=128  rows [c x (i=q,j=0) | c x (i=q,j=1)]
      s0..s2: K=64   rows  c x (s,2)  (ws blob, rows 0:64)
    All matmuls sit at row base 0 (mixed-base accumulation groups crash TRN2).
  * 6 matmuls / 6 ldweights per pixel pair (384 per core), 128-col
    stationaries, N=128 moving.  PSUM readout alternates vector/scalar by
    pair so the two engines touch different PSUM banks.
  * DMA budget is the binding resource (~340 GB/s/core aggregate under
    8-core load): total moved = 3.34 (x) + 9.44 (w) + 1.05 (out bf16) MB.
    Weight stream owns the sync HWDGE queue (nothing ever blocks it), x is
    on scalar, output DMAs on gpsimd.
  * Output bf16 in [psum-partition, pair, b] DRAM layout (contiguous DMA);
    host transposes to (B, O, H, W) fp32.
"""

import numpy as np
import ml_dtypes

BF16 = ml_dtypes.bfloat16

B, C, O, H, W, KS = 64, 64, 64, 32, 32, 3
L = H * W
NCORES = 8
RPC = H // NCORES            # image rows per core = 4
LC = RPC * W                 # pixels per core = 128
NP = LC // 2                 # pixel pairs per core = 64
HALO = RPC + 2               # 6 slab rows
WP = W + 2                   # padded width 34
PG = 8                       # pairs per weight DMA group
NG = NP // PG                # weight groups = 8

_CACHE = {}


def _build_nc():
    import concourse.bass as bass
    import concourse.bacc as bacc
    import concourse.tile as tile
    from concourse import mybir

    f32 = mybir.dt.float32
    bf16 = mybir.dt.bfloat16
    nc = bacc.Bacc(
        "TRN2", target_bir_lowering=False, debug=False, num_devices=NCORES
    )
    x_d = nc.dram_tensor("x", [128, HALO, B, WP], bf16, kind="ExternalInput")
    wq_d = nc.dram_tensor("wq", [128, NP, 3, 128], bf16, kind="ExternalInput")
    ws_d = nc.dram_tensor("ws", [64, NP, 3, 128], bf16, kind="ExternalInput")
    o_d = nc.dram_tensor("out", [128, NP, B], bf16, kind="ExternalOutput")

    with tile.TileContext(nc) as tc:
        with (
            tc.tile_pool(name="x1", bufs=1) as x1pool,
            tc.tile_pool(name="wq", bufs=4) as wpool,
            tc.tile_pool(name="ws", bufs=4) as spool,
            tc.tile_pool(name="orow", bufs=2) as opool,
            tc.tile_pool(name="ps", bufs=7, space=bass.MemorySpace.PSUM) as pspool,
            tc.tile_pool(name="psw", bufs=1, space=bass.MemorySpace.PSUM) as pswpool,
        ):
            t1 = x1pool.tile([128, HALO * B * WP], bf16)
            t1r = t1[:].rearrange("p (r b w) -> p r b w", r=HALO, b=B)
            # row-major slab: row 0 lands first (pair 0's q0 only needs it),
            # compute starts under the tail of the x transfer.
            nc.scalar.dma_start(t1r[:, 0:1], x_d[:, 0:1])
            nc.scalar.dma_start(t1r[:, 1:2], x_d[:, 1:2])
            nc.scalar.dma_start(t1r[:, 2:3], x_d[:, 2:3])
            nc.scalar.dma_start(t1r[:, 3:6], x_d[:, 3:6])

            # PE warmup: dummy matmuls while the first DMAs stream, so the
            # HAM clock gate is at 8/8 when the real matmul stream begins.
            scr = x1pool.tile([128, 256], bf16)
            nc.vector.memzero(scr[:])
            psw = pswpool.tile([64, 256], f32)
            for _ in range(40):
                nc.tensor.matmul(psw[:], scr[:, 0:64], scr[:], start=True, stop=True)

            # variable-size weight groups: small head (compute starts sooner)
            # and small tail (last output flushes sooner)
            sizes = [4, 4, 8, 8, 8, 8, 8, 8, 4, 4]
            t0g = 0
            for cnt in sizes:
                g0, g1 = t0g, t0g + cnt
                t0g = g1
                wt = wpool.tile([128, cnt, 3, 128], bf16)
                st = spool.tile([64, cnt, 3, 128], bf16)
                nc.sync.dma_start(wt[:], wq_d[:, g0:g1])
                nc.sync.dma_start(st[:], ws_d[:, g0:g1])
                orow = opool.tile([128, cnt, B], bf16)
                for tt in range(cnt):
                    t = g0 + tt
                    y, x0 = (2 * t) // W, (2 * t) % W
                    ps = pspool.tile([128, B, 2], f32)
                    for q in range(3):
                        nc.tensor.matmul(
                            ps[:], wt[:, tt, q, :],
                            t1r[:, y + q, :, x0 : x0 + 2],
                            start=(q == 0), stop=False,
                        )
                    for s in range(3):
                        nc.tensor.matmul(
                            ps[:], st[:, tt, s, :],
                            t1r[0:64, y + s, :, x0 + 2 : x0 + 4],
                            start=False, stop=(s == 2),
                        )
                    if tt % 2 == 0:
                        nc.vector.tensor_copy(orow[0:64, tt, :], ps[0:64, :, 0])
                        nc.vector.tensor_copy(orow[64:128, tt, :], ps[64:128, :, 1])
                    else:
                        nc.scalar.copy(orow[0:64, tt, :], ps[0:64, :, 0])
                        nc.scalar.copy(orow[64:128, tt, :], ps[64:128, :, 1])
                nc.scalar.dma_start(o_d[:, g0:g1, :], orow[:])
    nc.compile()
    return nc


def _get_nc():
    if "nc" not in _CACHE:
        _CACHE["nc"] = _build_nc()
    return _CACHE["nc"]


def _pack_x(x):
    """Per core: [128, HALO, B, WP] bf16 = [slab | slab shifted +1 col],
    row-major so row blocks stream independently."""
    xpad = np.pad(x, ((0, 0), (0, 0), (1, 1), (1, 1)))
    xpad = np.ascontiguousarray(xpad.transpose(1, 0, 2, 3))  # [C, B, 34, 34]
    outs = []
    for r in range(NCORES):
        slab = xpad[:, :, RPC * r : RPC * r + HALO, :]       # [C, B, 6, 34]
        slab = slab.transpose(0, 2, 1, 3)                    # [C, 6, B, 34]
        up = np.zeros_like(slab)
        up[..., : WP - 1] = slab[..., 1:]
        t1 = np.concatenate([slab, up], axis=0).astype(BF16)
        outs.append(np.ascontiguousarray(t1))
    return outs


def _pack_w(weight):
    """Chunked-contraction weight blobs, already in SBUF layout.

    wq: [core, NG, p=(j, c), tt, q, m=(e, o)]   (pair chunks, shifts (q, j))
    ws: [core, NG, c, tt, s, m=(e, o)]          (singles, shifts (s, 2))
    """
    w5 = weight.reshape(O, C, KS, KS, L)
    low = np.stack([w5[:, :, 0, 0], w5[:, :, 1, 0], w5[:, :, 2, 0]], axis=0)
    up = np.stack([w5[:, :, 0, 1], w5[:, :, 1, 1], w5[:, :, 2, 1]], axis=0)
    wq = np.stack([low, up], axis=1)          # [q, j, O, C, L]
    wq = wq.reshape(3, 2, O, C, NCORES, NP, 2)
    # -> [core, j, c, t, q, e, o]
    wq = wq.transpose(4, 1, 3, 5, 0, 6, 2)
    wq = np.ascontiguousarray(wq, dtype=BF16).reshape(NCORES, 128, NP, 3, 128)

    ws = np.stack([w5[:, :, 0, 2], w5[:, :, 1, 2], w5[:, :, 2, 2]], axis=0)
    ws = ws.reshape(3, O, C, NCORES, NP, 2)
    ws = ws.transpose(3, 2, 4, 0, 5, 1)       # [core, c, t, s, e, o]
    ws = np.ascontiguousarray(ws, dtype=BF16).reshape(NCORES, 64, NP, 3, 128)
    return wq, ws


def kernel(x, weight, bias, _trace=False, _trace_kwargs=None):
    from concourse.bass_utils import run_bass_kernel_spmd

    x = np.asarray(x, dtype=np.float32)
    weight = np.asarray(weight, dtype=np.float32)
    bias = np.asarray(bias, dtype=np.float32)

    nc = _get_nc()
    xs = _pack_x(x)
    wq, ws = _pack_w(weight)
    in_maps = [
        {"x": xs[r], "wq": wq[r], "ws": ws[r]} for r in range(NCORES)
    ]
    res = run_bass_kernel_spmd(
        nc, in_maps, list(range(NCORES)),
        trace=_trace, **(_trace_kwargs or {}),
    )
    # out[r]: [p=(e,o), t, b] bf16 -> [b, o, l=128r+2t+e]
    parts = []
    for r in range(NCORES):
        arr = res.results[r]["out"].astype(np.float32)
        arr = arr.reshape(2, O, NP, B).transpose(3, 1, 2, 0)  # [b, o, t, e]
        parts.append(arr.reshape(B, O, LC))
    out = np.concatenate(parts, axis=2).reshape(B, O, H, W)
    if np.any(bias):
        out = out + bias.reshape(1, O, H, W)
    if _trace:
        _CACHE["last_result"] = res
    return np.ascontiguousarray(out.astype(np.float32))
